# revision 23
# baseline (speedup 1.0000x reference)
"""MLLA block on 8 Trainium2 NeuronCores — v2.

Sequence-parallel over image rows (8 rows of the 64x64 token map per core),
halos via overlapping input windows; one AllReduce for the linear-attention
kv/ksum reduction; weights replicated per core.

Key design vs v1:
- Depthwise convs stay on the PE as diagonal-weight matmuls, but read their
  inputs with +-1 column offsets directly (3D access patterns skip the
  W-boundary wrap columns) — no shifted copies. Diagonal weight tiles are
  built on-chip (Act engine: Copy(ident * w_col)) from dense weights.
- q/k output features are de-interleaved per head host-side, so RoPE runs
  as contiguous-block tensor ops at DVE 2x rate.
- All tensor-engine transposes (v tiles, q_rope tiles, z) are XBAR DMA
  transposes — no PE time, no PSUM round-trips.
- z = 1/(q . k_mean) is applied after the a-matmul in F-layout via a
  12->128 selector-matmul broadcast of the reciprocal.
- MLP weights streamed as per-m [128,768] bf16 chunks (48 DMAs), 24
  software-pipelined PE iterations.
- All inputs host-packed contiguous per partition row.
"""
import numpy as np
import ml_dtypes

import concourse.bacc as bacc
import concourse.mybir as mybir
import concourse.tile as tile
from concourse.bass_utils import run_bass_kernel_spmd

dt = mybir.dt
AL = mybir.AluOpType
AF = mybir.ActivationFunctionType
BF = ml_dtypes.bfloat16

H = W = 64
L = H * W
C = 768
NH = 12
D = 64
NC = 8
KCH = 6
NTT = 5
T_X, T_X1, T_A, T_OUT = 896, 768, 640, 512

# tap order: full-width (dx=0) first and last so PSUM start/stop cover all
# columns; partial (dx=+-1) taps accumulate column subsets in between.
TAPS = [(-1, 0), (-1, -1), (-1, 1), (0, -1), (0, 1), (1, -1), (1, 1),
        (0, 0), (1, 0)]

# smf (small f32 constants) column layout
SMF_CW = 0          # 162: conv weights [27*k + 9*conv + tap]
SMF_CB = 162        # 18: conv biases [3*k + conv]
SMF_N1 = 180        # 12: ln1 w/b [2*k, 2*k+1]
SMF_N2 = 192        # 12
SMF_F1B = 204       # 24: fc1 bias col m
SMF_F2B = 228       # 6: fc2 bias col k
SMF_TOT = 234


DBG = False


def build_nc(qkb_nz, ln1_nz, ln2_nz, cb_nz):
    nc = bacc.Bacc("TRN2", target_bir_lowering=False, debug=False,
                   num_devices=NC)

    def din(name, shape, dtype=dt.float32):
        return nc.dram_tensor(name, list(shape), dtype, kind="ExternalInput")

    xp_d = din("xp", (128, KCH * T_X))
    cs_d = din("cs", (128, NTT * 768), dt.bfloat16)
    masks_d = din("masks", (128, 4))
    qkw_d = din("qkw", (128, KCH * 1536), dt.bfloat16)
    qkb_d = din("qkb", (1, 1536), dt.bfloat16)
    smf_d = din("smf", (128, SMF_TOT))
    smb_d = din("smb", (128, 128 + 768), dt.bfloat16)
    fc1w_d = din("fc1w", (128, 24 * 768), dt.bfloat16)
    fc2w_d = din("fc2w", (128, 24 * 768), dt.bfloat16)
    out_d = nc.dram_tensor("out", [128, KCH * T_OUT], dt.float32,
                           kind="ExternalOutput")
    if DBG:
        dbg_x1 = nc.dram_tensor("dbg_x1", [128, KCH * T_X1], dt.float32,
                                kind="ExternalOutput")
        dbg_xpre = nc.dram_tensor("dbg_xpre", [128, KCH * T_X1], dt.float32,
                                  kind="ExternalOutput")
        dbg_qe = nc.dram_tensor("dbg_qe", [128, NTT * C], dt.float32,
                                kind="ExternalOutput")
        dbg_kvks = nc.dram_tensor("dbg_kvks", [65, C], dt.float32,
                                  kind="ExternalOutput")
        dbg_kr = nc.dram_tensor("dbg_kr", [128, C], dt.float32,
                                kind="ExternalOutput")
        dbg_kvt = nc.dram_tensor("dbg_kvt", [65, NTT * C], dt.float32,
                                 kind="ExternalOutput")
        dbg_x1t = nc.dram_tensor("dbg_x1t", [128, C], dt.float32,
                                 kind="ExternalOutput")
        dbg_qrf = nc.dram_tensor("dbg_qrf", [128, KCH * T_A], dt.float32,
                                 kind="ExternalOutput")
        dbg_x2 = nc.dram_tensor("dbg_x2", [128, KCH * T_A], dt.float32,
                                kind="ExternalOutput")
        dbg_zt = nc.dram_tensor("dbg_zt", [128, NTT * 128], dt.float32,
                                kind="ExternalOutput")
        dbg_x3 = nc.dram_tensor("dbg_x3", [128, KCH * T_OUT], dt.float32,
                                kind="ExternalOutput")

    tcm = tile.TileContext(nc)
    tc = tcm.__enter__()

    def pool(name, bufs=1, space="SBUF", side=None):
        kw = {} if side is None else dict(side=side)
        cm = tc.tile_pool(name=name, bufs=bufs, space=space, **kw)
        return cm, cm.__enter__()

    # ---- SBUF pools. Left stack (LIFO; latest-closing opened first) ----
    cp_cm, cp = pool("const")
    wk_cm, wk = pool("work", bufs=2)
    dg_cm, dgp = pool("diag", bufs=4)
    misc_cm, miscp = pool("misc")
    dram_cm, dram = pool("dram", space="DRAM")
    f1s_cm, f1sp = pool("f1s", bufs=3)
    f2s_cm, f2sp = pool("f2s", bufs=3)
    ob_cm, obp = pool("ob")
    x3_cm, x3p = pool("x3")
    y_cm, yp = pool("y")
    hb_cm, hbp = pool("hb", bufs=2)
    x2_cm, x2p = pool("x2")
    csp_cm, csp = pool("csp")
    qkw_cm, qkwp = pool("qkw")
    # ---- right stack: qrF / qe / x-input / x1 (close P4/P5) ----
    qrf_cm, qrfp = pool("qrf", side="right")
    qe_cm, qep = pool("qe", side="right")
    xpp_cm, xpp = pool("xpp", side="right")
    x1_cm, x1p = pool("x1", side="right")

    # ---------------- prefetch ----------------
    t_smf = cp.tile([128, SMF_TOT], dt.float32, tag="smf", name="smf")
    nc.sync.dma_start(t_smf[:], smf_d[:, :])
    t_smb = cp.tile([128, 128 + 768], dt.bfloat16, tag="smb", name="smb")
    nc.sync.dma_start(t_smb[:], smb_d[:, :])
    t_masks = cp.tile([128, 4], dt.float32, tag="masks", name="masks")
    nc.sync.dma_start(t_masks[:], masks_d[:, :])
    t_xp = xpp.tile([128, KCH * T_X], dt.float32, tag="xp", name="xp")
    for k in range(KCH):
        nc.sync.dma_start(t_xp[:, T_X * k:T_X * (k + 1)],
                          xp_d[:, T_X * k:T_X * (k + 1)])
    t_cs = csp.tile([128, NTT * 768], dt.bfloat16, tag="cs", name="cs")
    nc.sync.dma_start(t_cs[:], cs_d[:, :])
    t_qkw = qkwp.tile([128, KCH * 1536], dt.bfloat16, tag="qkw", name="qkw")
    for i in range(2):
        nc.sync.dma_start(t_qkw[:, 4608 * i:4608 * (i + 1)],
                          qkw_d[:, 4608 * i:4608 * (i + 1)])
    if qkb_nz:
        t_qkb = cp.tile([1, 1536], dt.bfloat16, tag="qkb", name="qkb")
        nc.sync.dma_start(t_qkb[:], qkb_d[:, :])

    ident = t_smb[:, 0:128]

    ones_cb = cp.tile([128, 1], dt.bfloat16, tag="ones_cb", name="ones_cb")
    nc.vector.memset(ones_cb[:], 1.0)
    ones_rb = cp.tile([1, 128], dt.bfloat16, tag="ones_rb", name="ones_rb")
    nc.vector.memset(ones_rb[:], 1.0)
    ones_cr = cp.tile([128, 1], dt.float32r, tag="ones_cr", name="ones_cr")
    nc.vector.memset(ones_cr[:].bitcast(dt.float32), 1.0)
    ones_rr = cp.tile([1, 128], dt.float32r, tag="ones_rr", name="ones_rr")
    nc.vector.memset(ones_rr[:].bitcast(dt.float32), 1.0)

    def cw(k, conv_i, j):
        col = SMF_CW + 27 * k + 9 * conv_i + j
        return t_smf[:, col:col + 1]

    def cbias(k, conv_i):
        col = SMF_CB + 3 * k + conv_i
        return t_smf[:, col:col + 1]

    def xw(k, lo, hi):
        return t_xp[:, T_X * k + lo:T_X * k + hi]

    _dgn = [0]

    def diag(wcol):
        dg = dgp.tile([128, 128], dt.bfloat16, tag="dg", name="dg", bufs=6)
        _dgn[0] += 1
        if _dgn[0] % 2 == 0:
            nc.scalar.activation(dg[:], ident, AF.Copy, scale=wcol)
        else:
            nc.vector.tensor_scalar(dg[:], ident, wcol, None, AL.mult)
        return dg

    def i_to_tap(i):
        dy, dx = TAPS[i]
        return (dy + 1) * 3 + (dx + 1)

    # one depthwise 3x3 conv for chunk k accumulated into psum `acc`
    # ([128, T]); out col j reads input-window col 64+j+64*dy+dx. src3 is
    # the (p, rows, 64) view of the input chunk (T/64 + 2 rows).
    def conv(acc, accv, src3, conv_i, k, T):
        R = T // 64
        for i, (dy, dx) in enumerate(TAPS):
            dg = diag(cw(k, conv_i, i_to_tap(i)))
            start, stop = (i == 0), (i == len(TAPS) - 1)
            r0 = 1 + dy
            if dx == 0:
                for s0, s1 in ((0, 512), (512, T)) if T > 512 else ((0, T),):
                    nc.tensor.matmul(
                        acc[:, s0:s1],
                        dg[:],
                        src3[:, r0 + s0 // 64:r0 + s1 // 64, :],
                        start=start, stop=stop)
            else:
                lo, hi = (1, 64) if dx == -1 else (0, 63)
                rsplit = ((0, 8), (8, R)) if R > 8 else ((0, R),)
                for ra, rb in rsplit:
                    nc.tensor.matmul(
                        accv[:, ra:rb, lo:hi],
                        dg[:],
                        src3[:, r0 + ra:r0 + rb, lo + dx:hi + dx],
                        start=False, stop=False)

    def ln_tail(mu_ap, sq_ap, T, pp):
        A = wk.tile([1, T], dt.float32, tag="lnA", name="lnA", bufs=1)
        nc.vector.tensor_scalar(A[:], mu_ap, 1.0 / C, None, AL.mult)
        B = wk.tile([1, T], dt.float32, tag="lnB", name="lnB", bufs=1)
        nc.vector.tensor_scalar(B[:], sq_ap, 1.0 / C, None, AL.mult)
        Ct = wk.tile([1, T], dt.float32, tag="lnC", name="lnC", bufs=1)
        nc.vector.tensor_mul(Ct[:], A[:], A[:])
        nc.vector.tensor_sub(B[:], B[:], Ct[:])
        eps = wk.tile([1, 1], dt.float32, tag="eps", name="eps", bufs=1)
        nc.vector.memset(eps[:], 1e-5)
        nc.scalar.activation(B[:], B[:], AF.Sqrt, bias=eps[:], scale=1.0)
        rstd = wk.tile([1, T], dt.float32r, tag="lnC", name="lnR", bufs=1)
        with nc.allow_low_precision(reason="f32r rounding for PE broadcast"):
            nc.vector.reciprocal(rstd[:], B[:])
        nmrt = wk.tile([1, T], dt.float32r, tag="lnB", name="lnN", bufs=1)
        nc.vector.scalar_tensor_tensor(nmrt[:], A[:], -1.0, rstd[:],
                                       AL.mult, AL.mult)
        rbc = pp.tile([128, T], dt.float32, tag=f"rbc{T}", name=f"rbc{T}")
        nbc = pp.tile([128, T], dt.float32, tag=f"nbc{T}", name=f"nbc{T}")
        for s0, s1 in ((0, 512), (512, T)) if T > 512 else ((0, T),):
            nc.tensor.matmul(rbc[:, s0:s1], ones_rr[:], rstd[:, s0:s1],
                             start=True, stop=True)
            nc.tensor.matmul(nbc[:, s0:s1], ones_rr[:], nmrt[:, s0:s1],
                             start=True, stop=True)
        rsb = miscp.tile([128, T], dt.bfloat16, tag=f"rsb{T}", name=f"rsb{T}")
        nc.vector.tensor_copy(rsb[:], rbc[:])
        nsb = miscp.tile([128, T], dt.bfloat16, tag=f"nsb{T}", name=f"nsb{T}")
        nc.vector.tensor_copy(nsb[:], nbc[:])
        return rsb, nsb

    # ======== P1: conv1 + x1pre + LN1 stats ========
    p1s_cm, p1s = pool("p1s", space="PSUM")
    p1c_cm, p1c = pool("p1c", bufs=2, space="PSUM")
    xbf_cm, xbfp = pool("xbf")
    xpre_cm, xprep = pool("xpre")

    t_xbf = xbfp.tile([128, KCH * T_X], dt.bfloat16, tag="xbf", name="xbf")
    for k in range(KCH):
        if k % 2 == 0:
            nc.vector.tensor_copy(t_xbf[:, T_X * k:T_X * (k + 1)],
                                  xw(k, 0, T_X))
        else:
            nc.scalar.copy(t_xbf[:, T_X * k:T_X * (k + 1)], xw(k, 0, T_X))

    mu1 = p1s.tile([1, T_X1], dt.float32, tag="mu1", name="mu1")
    sq1 = p1s.tile([1, T_X1], dt.float32, tag="sq1", name="sq1")
    t_xpre = xprep.tile([128, KCH * T_X1], dt.bfloat16, tag="xpre",
                        name="xpre")
    src3_all = t_xbf[:].rearrange("p (g w) -> p g w", w=64)
    stats = []

    def emit_stats1(k):
        xpk, sqk = stats[k]
        for s0, s1 in ((0, 512), (512, T_X1)):
            nc.tensor.matmul(mu1[0:1, s0:s1], ones_cb[:], xpk[:, s0:s1],
                             start=(k == 0), stop=(k == KCH - 1))
            nc.tensor.matmul(sq1[0:1, s0:s1], ones_cb[:], sqk[:, s0:s1],
                             start=(k == 0), stop=(k == KCH - 1))

    for k in range(KCH):
        cv = p1c.tile([128, T_X1], dt.float32, tag="cv", name="cv")
        cvv = cv[:].rearrange("p (r w) -> p r w", w=64)
        conv(cv[:], cvv, src3_all[:, 14 * k:14 * (k + 1), :], 0, k, T_X1)
        xpk = t_xpre[:, T_X1 * k:T_X1 * (k + 1)]
        if cb_nz:
            nc.vector.scalar_tensor_tensor(xpk, cv[:], cbias(k, 0),
                                           xw(k, 64, 64 + T_X1),
                                           AL.add, AL.add)
        else:
            nc.vector.tensor_add(xpk, cv[:], xw(k, 64, 64 + T_X1))
        sqk = wk.tile([128, T_X1], dt.bfloat16, tag="w768b", name="sqk",
                      bufs=2)
        if k % 2 == 0:
            nc.gpsimd.tensor_mul(sqk[:], xpk, xpk)
        else:
            nc.scalar.square(sqk[:], xpk)
        stats.append((xpk, sqk))
        if k > 0:
            emit_stats1(k - 1)
    emit_stats1(KCH - 1)
    p1c_cm.__exit__(None, None, None)

    # ======== P2: LN1 normalize -> x1 ========
    p2_cm, p2 = pool("p2", space="PSUM")
    rsb1, nsb1 = ln_tail(mu1[0:1, :], sq1[0:1, :], T_X1, p2)
    t_x1 = x1p.tile([128, KCH * T_X1], dt.bfloat16, tag="x1", name="x1")
    for k in range(KCH):
        xpk = t_xpre[:, T_X1 * k:T_X1 * (k + 1)]
        x1k = t_x1[:, T_X1 * k:T_X1 * (k + 1)]
        tmp = wk.tile([128, T_X1], dt.bfloat16, tag="w768b", name="tmp",
                      bufs=2)
        nc.vector.tensor_mul(tmp[:], xpk, rsb1[:])
        if ln1_nz:
            tmp2 = wk.tile([128, T_X1], dt.bfloat16, tag="w768c",
                           name="tmp2")
            nc.vector.tensor_add(tmp2[:], tmp[:], nsb1[:])
            nc.vector.tensor_scalar(
                x1k, tmp2[:],
                t_smf[:, SMF_N1 + 2 * k:SMF_N1 + 2 * k + 1],
                t_smf[:, SMF_N1 + 2 * k + 1:SMF_N1 + 2 * k + 2],
                AL.mult, AL.add)
        else:
            nc.vector.tensor_add(x1k, tmp[:], nsb1[:])
        nc.vector.tensor_scalar(x1k[:, 0:128], x1k[:, 0:128],
                                t_masks[:, 0:1], None, AL.mult)
        nc.vector.tensor_scalar(x1k[:, 640:768], x1k[:, 640:768],
                                t_masks[:, 1:2], None, AL.mult)
    def dump(dst, src_ap, T):
        for kk in range(0, src_ap.shape[1], T):
            f = wk.tile([src_ap.shape[0], T], dt.float32, tag="dumpf",
                        name="dumpf", bufs=2)
            nc.vector.tensor_copy(f[:], src_ap[:, kk:kk + T])
            nc.scalar.dma_start(dst[:, kk:kk + T], f[:])

    if DBG:
        dump(dbg_x1, t_x1[:], T_X1)
        dump(dbg_xpre, t_xpre[:], T_X1)
    xpre_cm.__exit__(None, None, None)
    xbf_cm.__exit__(None, None, None)
    p2_cm.__exit__(None, None, None)
    p1s_cm.__exit__(None, None, None)

    # ======== P3: qk + elu + rope + v/qr transposes + kv/ks ========
    p3k_cm, p3k = pool("p3k", space="PSUM")
    p3q_cm, p3q = pool("p3q", bufs=2, space="PSUM")
    x1t_cm, x1tp = pool("x1t", bufs=2)
    kr_cm, krp = pool("kr", bufs=2)

    kvks = p3k.tile([65, C], dt.float32, tag="kvks", name="kvks")
    t_qe = qep.tile([128, NTT * C], dt.bfloat16, tag="qe", name="qe")
    t_qrF = qrfp.tile([128, KCH * T_A], dt.bfloat16, tag="qrF", name="qrF")

    def rope(dst, src, t):
        sv = src.rearrange("p (h d) -> p h d", d=64)
        dv = dst.rearrange("p (h d) -> p h d", d=64)
        ct3 = t_cs[:, 768 * t:768 * t + 384].rearrange(
            "p (h q) -> p h q", q=32)
        st3 = t_cs[:, 768 * t + 384:768 * t + 768].rearrange(
            "p (h q) -> p h q", q=32)
        E, O = sv[:, :, 0:32], sv[:, :, 32:64]
        ms = []
        for mi, (a, b) in enumerate(((E, ct3), (O, st3), (O, ct3), (E, st3))):
            m = wk.tile([128, 384], dt.bfloat16, tag=f"m{mi}", name=f"m{mi}",
                        bufs=1)
            eng = nc.vector if mi % 2 == 0 else nc.gpsimd
            eng.tensor_mul(m[:].rearrange("p (h q) -> p h q", q=32), a, b)
            ms.append(m[:].rearrange("p (h q) -> p h q", q=32))
        nc.vector.tensor_sub(dv[:, :, 0:32], ms[0], ms[1])
        nc.vector.tensor_add(dv[:, :, 32:64], ms[2], ms[3])

    def emit_kv(t, kr, ke):
        for h in range(NH):
            nc.tensor.matmul(kvks[0:64, 64 * h:64 * (h + 1)],
                             kr[:, 64 * h:64 * (h + 1)],
                             x1t_tiles[t][:, 64 * h:64 * (h + 1)],
                             start=(t == 0), stop=(t == NTT - 1))
        for s0, s1 in ((0, 512), (512, C)):
            nc.tensor.matmul(kvks[64:65, s0:s1], ones_cb[:], ke[:, s0:s1],
                             start=(t == 0), stop=(t == NTT - 1))

    x1t_tiles = {}
    for t in range(NTT):
        tok0 = 64 + 128 * t
        qkps = p3q.tile([128, 1536], dt.float32, tag="qkps", name="qkps")
        for k in range(KCH):
            for s in range(3):
                nc.tensor.matmul(
                    qkps[:, 512 * s:512 * (s + 1)],
                    t_x1[:, T_X1 * k + tok0:T_X1 * k + tok0 + 128],
                    t_qkw[:, 1536 * k + 512 * s:1536 * k + 512 * (s + 1)],
                    start=(k == 0), stop=(k == KCH - 1 and not qkb_nz))
        if qkb_nz:
            for s in range(3):
                nc.tensor.matmul(qkps[:, 512 * s:512 * (s + 1)],
                                 ones_rb[:],
                                 t_qkb[:, 512 * s:512 * (s + 1)],
                                 start=False, stop=True)
        qe = t_qe[:, C * t:C * (t + 1)]
        ke = krp.tile([128, C], dt.bfloat16, tag="ke", name="ke")
        for s in range(3):
            ex = wk.tile([128, 512], dt.bfloat16, tag="ex", name="ex")
            nc.scalar.activation(ex[:], qkps[:, 512 * s:512 * (s + 1)],
                                 AF.Exp)
            rl = wk.tile([128, 512], dt.bfloat16, tag="rl", name="rl")
            nc.scalar.activation(rl[:], qkps[:, 512 * s:512 * (s + 1)],
                                 AF.Relu)
            if s == 0:
                nc.vector.scalar_tensor_tensor(qe[:, 0:512], ex[:], 1.0,
                                               rl[:], AL.min, AL.add)
            elif s == 1:
                nc.vector.scalar_tensor_tensor(qe[:, 512:768], ex[:, 0:256],
                                               1.0, rl[:, 0:256],
                                               AL.min, AL.add)
                nc.vector.scalar_tensor_tensor(ke[:, 0:256], ex[:, 256:512],
                                               1.0, rl[:, 256:512],
                                               AL.min, AL.add)
            else:
                nc.vector.scalar_tensor_tensor(ke[:, 256:768], ex[:], 1.0,
                                               rl[:], AL.min, AL.add)
        # mask halo rows so kv/ks count each token exactly once globally
        if t == 0:
            nc.gpsimd.memset(ke[0:64, :], 0.0)
        if t == NTT - 1:
            nc.gpsimd.memset(ke[64:128, :], 0.0)
        kr = krp.tile([128, C], dt.bfloat16, tag="kr", name="kr")
        rope(kr[:], ke[:], t)
        if DBG and t == 1:
            krf = wk.tile([128, C], dt.float32, tag="dumpf", name="krf",
                          bufs=2)
            nc.vector.tensor_copy(krf[:], kr[:])
            nc.scalar.dma_start(dbg_kr[:, :], krf[:])
        qr = krp.tile([128, C], dt.bfloat16, tag="qr", name="qr")
        rope(qr[:], qe, t)
        x1t = x1tp.tile([128, C], dt.bfloat16, tag="x1t", name="x1t")
        x1t_tiles[t] = x1t
        for k in range(KCH):
            nc.sync.dma_start_transpose(
                x1t[:, 128 * k:128 * (k + 1)],
                t_x1[:, T_X1 * k + tok0:T_X1 * k + tok0 + 128])
        if DBG and t == 1:
            x1tf = wk.tile([128, C], dt.float32, tag="dumpf", name="x1tf",
                           bufs=2)
            nc.vector.tensor_copy(x1tf[:], x1t[:])
            nc.scalar.dma_start(dbg_x1t[:, :], x1tf[:])
        for k in range(KCH):
            nc.sync.dma_start_transpose(
                t_qrF[:, T_A * k + 128 * t:T_A * k + 128 * (t + 1)],
                qr[:, 128 * k:128 * (k + 1)])
        emit_kv(t, kr, ke)
        if DBG:
            kvf = wk.tile([65, C], dt.float32, tag="dumpf65", name="kvf",
                          bufs=2)
            nc.vector.tensor_copy(kvf[:], kvks[:])
            nc.scalar.dma_start(dbg_kvt[:, C * t:C * (t + 1)], kvf[:])

    if DBG:
        dump(dbg_qe, t_qe[:], C)
        dump(dbg_qrf, t_qrF[:], T_A)
        kvsb = wk.tile([65, C], dt.float32, tag="dumpf65", name="kvsb",
                       bufs=2)
        nc.vector.tensor_copy(kvsb[:], kvks[:])
        nc.scalar.dma_start(dbg_kvks[:, :], kvsb[:])

    # ======== collective ========
    ccsb = miscp.tile([65, C], dt.float32, tag="ccsb", name="ccsb")
    nc.vector.tensor_copy(ccsb[:], kvks[:])
    cc_in = dram.tile([65, C], dt.float32, tag="cc_in", name="cc_in")
    cc_out = dram.tile([65, C], dt.float32, tag="cc_out", name="cc_out")
    nc.gpsimd.dma_start(cc_in[:, :], ccsb[:])
    nc.gpsimd.collective_compute(
        "AllReduce", AL.add, replica_groups=[list(range(NC))],
        ins=[cc_in[:].opt()], outs=[cc_out[:].opt()])

    kr_cm.__exit__(None, None, None)
    x1t_cm.__exit__(None, None, None)
    p3q_cm.__exit__(None, None, None)
    p3k_cm.__exit__(None, None, None)

    # ======== P4: lepe conv (overlaps the collective) ========
    p4l_cm, p4l = pool("p4l", bufs=2, space="PSUM")
    t_x2 = x2p.tile([128, KCH * T_A], dt.float32, tag="x2", name="x2")
    x13 = t_x1[:].rearrange("p (g w) -> p g w", w=64)
    for k in range(KCH):
        lp = p4l.tile([128, T_A], dt.float32, tag="lp", name="lp")
        lpv = lp[:].rearrange("p (r w) -> p r w", w=64)
        conv(lp[:], lpv, x13[:, 12 * k:12 * (k + 1), :], 1, k, T_A)
        x2k = t_x2[:, T_A * k:T_A * (k + 1)]
        if cb_nz:
            nc.vector.scalar_tensor_tensor(x2k, lp[:], cbias(k, 1),
                                           xw(k, 128, 128 + T_A),
                                           AL.add, AL.add)
        else:
            nc.vector.tensor_add(x2k, lp[:], xw(k, 128, 128 + T_A))
    p4l_cm.__exit__(None, None, None)

    # fetch collective results
    kvg_f = miscp.tile([128, C], dt.float32, tag="kvg_f", name="kvg_f")
    nc.gpsimd.dma_start(kvg_f[0:64, :], cc_out[0:64, :])
    nc.gpsimd.dma_start(kvg_f[64:128, :], cc_out[0:64, :])
    ksg_f = miscp.tile([1, C], dt.float32, tag="ccsb", name="ksg_f")
    nc.gpsimd.dma_start(ksg_f[:], cc_out[64:65, :])
    kvg = miscp.tile([128, C], dt.bfloat16, tag="kvg", name="kvg")
    nc.vector.tensor_copy(kvg[:], kvg_f[:])
    ksg_b = wk.tile([1, C], dt.bfloat16, tag="ksg_b", name="ksg_b", bufs=1)
    nc.vector.tensor_copy(ksg_b[:], ksg_f[:])
    p4k_cm, p4k = pool("p4k", space="PSUM")
    ksbc = p4k.tile([128, C], dt.float32, tag="ksbc", name="ksbc")
    for s0, s1 in ((0, 512), (512, C)):
        nc.tensor.matmul(ksbc[:, s0:s1], ones_rb[:], ksg_b[:, s0:s1],
                         start=True, stop=True)
    ksb = miscp.tile([128, C], dt.bfloat16, tag="ksb", name="ksb")
    nc.vector.tensor_copy(ksb[:], ksbc[:])
    p4k_cm.__exit__(None, None, None)

    # ======== P5: z + a, x2 += a*z ========
    zpad = miscp.tile([128, NTT * 128], dt.bfloat16, tag="zpad", name="zpad")
    nc.gpsimd.memset(zpad[:], 0.0)
    zT = miscp.tile([128, NTT * 128], dt.bfloat16, tag="zT", name="zT")
    for t in range(NTT):
        zd = wk.tile([128, C], dt.bfloat16, tag="zd", name="zd", bufs=2)
        eng = nc.vector if t % 2 == 0 else nc.gpsimd
        eng.tensor_mul(zd[:], t_qe[:, C * t:C * (t + 1)], ksb[:])
        zs = wk.tile([128, NH], dt.float32, tag="zs", name="zs")
        nc.vector.tensor_reduce(zs[:],
                                zd[:].rearrange("p (h d) -> p h d", d=D),
                                mybir.AxisListType.X, AL.add)
        zv = wk.tile([128, NH], dt.float32, tag="zv", name="zv")
        nc.vector.tensor_scalar(zv[:], zs[:], 4096e-6, None, AL.add)
        zr = wk.tile([128, NH], dt.float32, tag="zr", name="zr")
        nc.vector.reciprocal(zr[:], zv[:])
        nc.vector.tensor_copy(zpad[:, 128 * t:128 * t + NH], zr[:])
        nc.sync.dma_start_transpose(zT[:, 128 * t:128 * (t + 1)],
                                    zpad[:, 128 * t:128 * (t + 1)])

    p5z_cm, p5z = pool("p5z", bufs=2, space="PSUM")
    p5a_cm, p5a = pool("p5a", bufs=2, space="PSUM")

    def emit_zf(k):
        zF = p5z.tile([128, T_A], dt.float32, tag="zF", name="zF")
        sel = t_smb[0:12, 128 + 128 * k:128 + 128 * (k + 1)]
        for s0, s1 in ((0, 512), (512, T_A)):
            nc.tensor.matmul(zF[:, s0:s1], sel, zT[0:12, s0:s1],
                             start=True, stop=True)
        return zF

    zF_k = emit_zf(0)
    for k in range(KCH):
        zF_next = emit_zf(k + 1) if k + 1 < KCH else None
        qzF = wk.tile([128, T_A], dt.bfloat16, tag="qzF", name="qzF")
        nc.vector.tensor_mul(qzF[:], t_qrF[:, T_A * k:T_A * (k + 1)],
                             zF_k[:])
        ap = p5a.tile([128, T_A], dt.float32, tag="ap", name="ap")
        for half in range(2):
            h = 2 * k + half
            off = 64 * half
            for s0, s1 in ((0, 512), (512, T_A)):
                nc.tensor.matmul(ap[off:off + 64, s0:s1],
                                 kvg[off:off + 64, 64 * h:64 * (h + 1)],
                                 qzF[off:off + 64, s0:s1],
                                 start=True, stop=True)
        x2k = t_x2[:, T_A * k:T_A * (k + 1)]
        nc.vector.tensor_add(x2k, x2k, ap[:])
        zF_k = zF_next
    for k in range(KCH):
        x2k = t_x2[:, T_A * k:T_A * (k + 1)]
        nc.vector.tensor_scalar(x2k[:, 0:64], x2k[:, 0:64],
                                t_masks[:, 2:3], None, AL.mult)
        nc.vector.tensor_scalar(x2k[:, 576:640], x2k[:, 576:640],
                                t_masks[:, 3:4], None, AL.mult)
    if DBG:
        dump(dbg_zt, zT[:], 128)
        dump(dbg_x2, t_x2[:], T_A)
    p5a_cm.__exit__(None, None, None)
    p5z_cm.__exit__(None, None, None)
    # right-stack pops (reverse of open order: x1, xpp, qe, qrf)
    x1_cm.__exit__(None, None, None)
    xpp_cm.__exit__(None, None, None)
    qe_cm.__exit__(None, None, None)
    qrf_cm.__exit__(None, None, None)
    # left-stack pops for P3 leftovers
    qkw_cm.__exit__(None, None, None)
    csp_cm.__exit__(None, None, None)

    # ======== P6: conv2 + x3 + LN2 ========
    xb2_cm, xb2p = pool("xb2")
    p6s_cm, p6s = pool("p6s", space="PSUM")
    p6c_cm, p6c = pool("p6c", bufs=2, space="PSUM")
    t_xb2 = xb2p.tile([128, KCH * T_A], dt.bfloat16, tag="xb2", name="xb2")
    for k in range(KCH):
        nc.scalar.copy(t_xb2[:, T_A * k:T_A * (k + 1)],
                       t_x2[:, T_A * k:T_A * (k + 1)])
    mu2 = p6s.tile([1, T_OUT], dt.float32, tag="mu2", name="mu2")
    sq2 = p6s.tile([1, T_OUT], dt.float32, tag="sq2", name="sq2")
    t_x3 = x3p.tile([128, KCH * T_OUT], dt.float32r, tag="x3", name="x3")
    xb23 = t_xb2[:].rearrange("p (g w) -> p g w", w=64)
    stats2 = []

    def emit_stats2(k):
        x3k, sqk = stats2[k]
        nc.tensor.matmul(mu2[0:1, :], ones_cr[:], x3k,
                         start=(k == 0), stop=(k == KCH - 1))
        nc.tensor.matmul(sq2[0:1, :], ones_cr[:], sqk[:],
                         start=(k == 0), stop=(k == KCH - 1))

    for k in range(KCH):
        c2 = p6c.tile([128, T_OUT], dt.float32, tag="c2", name="c2")
        c2v = c2[:].rearrange("p (r w) -> p r w", w=64)
        conv(c2[:], c2v, xb23[:, 10 * k:10 * (k + 1), :], 2, k, T_OUT)
        x3k = t_x3[:, T_OUT * k:T_OUT * (k + 1)]
        if cb_nz:
            nc.vector.scalar_tensor_tensor(
                x3k, c2[:], cbias(k, 2),
                t_x2[:, T_A * k + 64:T_A * k + 64 + T_OUT], AL.add, AL.add)
        else:
            nc.vector.tensor_add(
                x3k, c2[:], t_x2[:, T_A * k + 64:T_A * k + 64 + T_OUT])
        sqk = wk.tile([128, T_OUT], dt.float32r, tag="sq2k", name="sq2k",
                      bufs=2)
        nc.scalar.square(sqk[:], x3k)
        stats2.append((x3k, sqk))
        if k > 0:
            emit_stats2(k - 1)
    emit_stats2(KCH - 1)
    if DBG:
        dump(dbg_x3, t_x3[:].bitcast(dt.float32), T_OUT)
    p6c_cm.__exit__(None, None, None)

    p6b_cm, p6b = pool("p6b", space="PSUM")
    rsb2, nsb2 = ln_tail(mu2[0:1, :], sq2[0:1, :], T_OUT, p6b)
    t_y = yp.tile([128, KCH * T_OUT], dt.bfloat16, tag="y", name="y")
    for k in range(KCH):
        x3k = t_x3[:, T_OUT * k:T_OUT * (k + 1)]
        yk = t_y[:, T_OUT * k:T_OUT * (k + 1)]
        tmp = wk.tile([128, T_OUT], dt.bfloat16, tag="w512b", name="tmpy",
                      bufs=2)
        eng = nc.vector if k % 2 == 0 else nc.gpsimd
        eng.tensor_mul(tmp[:], x3k, rsb2[:])
        if ln2_nz:
            tmp2 = wk.tile([128, T_OUT], dt.bfloat16, tag="w512c",
                           name="tmpy2")
            eng.tensor_add(tmp2[:], tmp[:], nsb2[:])
            nc.vector.tensor_scalar(
                yk, tmp2[:],
                t_smf[:, SMF_N2 + 2 * k:SMF_N2 + 2 * k + 1],
                t_smf[:, SMF_N2 + 2 * k + 1:SMF_N2 + 2 * k + 2],
                AL.mult, AL.add)
        else:
            eng.tensor_add(yk, tmp[:], nsb2[:])
    p6b_cm.__exit__(None, None, None)
    p6s_cm.__exit__(None, None, None)
    xb2_cm.__exit__(None, None, None)
    x2_cm.__exit__(None, None, None)

    # ======== P7: MLP (streamed per-m weight chunks) ========
    p7h_cm, p7h = pool("p7h", bufs=2, space="PSUM")
    p7o_cm, p7o = pool("p7o", space="PSUM")
    ops = [p7o.tile([128, T_OUT], dt.float32, tag=f"op{k}", name=f"op{k}")
           for k in range(KCH)]

    def emit_fc1(m):
        w1 = f1sp.tile([128, 768], dt.bfloat16, tag="w1", name="w1")
        nc.sync.dma_start(w1[:], fc1w_d[:, 768 * m:768 * (m + 1)])
        hp = p7h.tile([128, T_OUT], dt.float32, tag="hp", name="hp")
        for k in range(KCH):
            nc.tensor.matmul(hp[:], w1[:, 128 * k:128 * (k + 1)],
                             t_y[:, T_OUT * k:T_OUT * (k + 1)],
                             start=(k == 0), stop=(k == KCH - 1))
        return hp

    hp = emit_fc1(0)
    for m in range(24):
        hb = hbp.tile([128, T_OUT], dt.bfloat16, tag="hb", name="hb")
        nc.scalar.activation(hb[:], hp[:], AF.Silu,
                             bias=t_smf[:, SMF_F1B + m:SMF_F1B + m + 1],
                             scale=1.0)
        if m < 23:
            hp = emit_fc1(m + 1)
        w2 = f2sp.tile([128, 768], dt.bfloat16, tag="w2", name="w2")
        nc.sync.dma_start(w2[:], fc2w_d[:, 768 * m:768 * (m + 1)])
        for k in range(KCH):
            nc.tensor.matmul(ops[k][:], w2[:, 128 * k:128 * (k + 1)],
                             hb[:], start=(m == 0), stop=(m == 23))
    for k in range(KCH):
        of = obp.tile([128, T_OUT], dt.float32, tag="of", name="of", bufs=2)
        nc.vector.scalar_tensor_tensor(
            of[:], ops[k][:],
            t_smf[:, SMF_F2B + k:SMF_F2B + k + 1],
            t_x3[:, T_OUT * k:T_OUT * (k + 1)].bitcast(dt.float32),
            AL.add, AL.add)
        nc.sync.dma_start(out_d[:, T_OUT * k:T_OUT * (k + 1)], of[:])

    p7o_cm.__exit__(None, None, None)
    p7h_cm.__exit__(None, None, None)
    for cm in (hb_cm, y_cm, x3_cm, ob_cm, f2s_cm, f1s_cm,
               dram_cm, misc_cm, dg_cm, wk_cm, cp_cm):
        cm.__exit__(None, None, None)
    tcm.__exit__(None, None, None)
    nc.finalize()
    return nc


# ----------------------------------------------------------------------------
# host side
# ----------------------------------------------------------------------------

_NC_CACHE = {}
_LAST_RES = None


def _rope_tables():
    k_max = C // 4
    theta = 1.0 / (10000.0 ** (np.arange(k_max, dtype=np.float64) / k_max))
    ax = np.arange(H, dtype=np.float64)[:, None, None] * theta
    ay = np.arange(W, dtype=np.float64)[None, :, None] * theta
    ang = np.concatenate([
        np.broadcast_to(ax, (H, W, k_max)),
        np.broadcast_to(ay, (H, W, k_max))], axis=-1).reshape(L, C // 2)
    return np.cos(ang).astype(np.float32), np.sin(ang).astype(np.float32)


def _chunk_major(a):
    # (C, N) -> (128, KCH*N) with col N*k+j = a[128k+p, j]
    n = a.shape[1]
    return np.ascontiguousarray(
        a.reshape(KCH, 128, n).transpose(1, 0, 2).reshape(128, KCH * n))


_PERM = None


def _perm():
    global _PERM
    if _PERM is None:
        p = np.zeros(2 * C, np.int64)
        for b in range(2 * C // 64):
            base = 64 * b
            p[base:base + 32] = base + 2 * np.arange(32)
            p[base + 32:base + 64] = base + 2 * np.arange(32) + 1
        _PERM = p
    return _PERM


def kernel(x, cpe1_w, cpe1_b, norm1_w, norm1_b, qk_w, qk_b, lepe_w, lepe_b,
           cpe2_w, cpe2_b, norm2_w, norm2_b, fc1_w, fc1_b, fc2_w, fc2_b):
    f32 = np.float32
    x = np.asarray(x, f32)
    cos_full, sin_full = _rope_tables()
    x_img = x.reshape(H, W, C)

    qkb_nz = bool(np.any(np.asarray(qk_b) != 0))
    ln1_nz = not (np.allclose(norm1_w, 1) and np.allclose(norm1_b, 0))
    ln2_nz = not (np.allclose(norm2_w, 1) and np.allclose(norm2_b, 0))
    cb_nz = bool(np.any(np.asarray(cpe1_b) != 0) or
                 np.any(np.asarray(lepe_b) != 0) or
                 np.any(np.asarray(cpe2_b) != 0))

    key = (qkb_nz, ln1_nz, ln2_nz, cb_nz)
    if key not in _NC_CACHE:
        _NC_CACHE[key] = build_nc(*key)
    nc = _NC_CACHE[key]

    # ---- shared packing ----
    smf = np.zeros((128, SMF_TOT), f32)
    for k in range(KCH):
        for ci, wnd in enumerate((cpe1_w, lepe_w, cpe2_w)):
            wnd = np.asarray(wnd, f32)
            for j in range(9):
                smf[:, SMF_CW + 27 * k + 9 * ci + j] = \
                    wnd[128 * k:128 * (k + 1), 0, j // 3, j % 3]
        for ci, bnd in enumerate((cpe1_b, lepe_b, cpe2_b)):
            smf[:, SMF_CB + 3 * k + ci] = \
                np.asarray(bnd, f32)[128 * k:128 * (k + 1)]
        smf[:, SMF_N1 + 2 * k] = np.asarray(norm1_w, f32)[128 * k:128 * (k + 1)]
        smf[:, SMF_N1 + 2 * k + 1] = np.asarray(norm1_b, f32)[128 * k:128 * (k + 1)]
        smf[:, SMF_N2 + 2 * k] = np.asarray(norm2_w, f32)[128 * k:128 * (k + 1)]
        smf[:, SMF_N2 + 2 * k + 1] = np.asarray(norm2_b, f32)[128 * k:128 * (k + 1)]
        smf[:, SMF_F2B + k] = np.asarray(fc2_b, f32)[128 * k:128 * (k + 1)]
    for m in range(24):
        smf[:, SMF_F1B + m] = np.asarray(fc1_b, f32)[128 * m:128 * (m + 1)]

    smb = np.zeros((128, 128 + 768), f32)
    smb[:, 0:128] = np.eye(128, dtype=f32)
    for k in range(KCH):
        for half in range(2):
            h = 2 * k + half
            smb[h, 128 + 128 * k + 64 * half:
                128 + 128 * k + 64 * (half + 1)] = 1.0

    perm = _perm()
    qkw_p = np.asarray(qk_w, f32)[perm, :]          # (1536, 768)
    qkw = _chunk_major(qkw_p.T)                      # (128, 6*1536)
    qkb = np.asarray(qk_b, f32)[perm].reshape(1, 1536)

    w1t = np.asarray(fc1_w, f32).T                   # (768, 3072)
    fc1w = np.ascontiguousarray(
        w1t.reshape(KCH, 128, 24, 128).transpose(1, 2, 0, 3).reshape(
            128, 24 * 768))
    w2t = np.asarray(fc2_w, f32).T                   # (3072, 768)
    fc2w = np.ascontiguousarray(
        w2t.reshape(24, 128, KCH, 128).transpose(1, 0, 2, 3).reshape(
            128, 24 * 768))

    shared = dict(
        smf=smf, smb=smb.astype(BF),
        qkw=qkw.astype(BF), qkb=qkb.astype(BF),
        fc1w=fc1w.astype(BF), fc2w=fc2w.astype(BF),
    )

    in_maps = []
    for c in range(NC):
        r0 = c * 8
        xe = np.zeros((14, W, C), f32)
        lo = max(0, r0 - 3)
        hi = min(H, r0 + 11)
        xe[lo - (r0 - 3):hi - (r0 - 3)] = x_img[lo:hi]
        xp = _chunk_major(np.ascontiguousarray(xe.reshape(T_X, C).T))

        t0 = (r0 - 1) * W
        idx = np.arange(t0, t0 + T_A)
        ok = (idx >= 0) & (idx < L)
        cch = np.zeros((T_A, C // 2), f32)
        ssh = np.zeros((T_A, C // 2), f32)
        cch[ok] = cos_full[idx[ok]]
        ssh[ok] = sin_full[idx[ok]]
        cs = np.zeros((128, NTT * 768), f32)
        for t in range(NTT):
            cs[:, 768 * t:768 * t + 384] = cch[128 * t:128 * (t + 1)]
            cs[:, 768 * t + 384:768 * t + 768] = ssh[128 * t:128 * (t + 1)]

        masks = np.ones((128, 4), f32)
        if c == 0:
            masks[:, 0] = 0.0
            masks[:, 2] = 0.0
        if c == NC - 1:
            masks[:, 1] = 0.0
            masks[:, 3] = 0.0

        in_maps.append(dict(xp=xp, cs=cs.astype(BF), masks=masks, **shared))

    res = run_bass_kernel_spmd(nc, in_maps, core_ids=list(range(NC)))
    global _LAST_RES
    _LAST_RES = res
    full = np.empty((L, C), np.float32)
    for c in range(NC):
        o = res.results[c]["out"].reshape(128, KCH, T_OUT)
        for k in range(KCH):
            full[T_OUT * c:T_OUT * (c + 1), 128 * k:128 * (k + 1)] = o[:, k].T
    return full


if __name__ == "__main__":
    import reference
    inputs = {k: np.asarray(v) for k, v in reference.setup_inputs().items()}
    exp = np.asarray(reference.reference(**reference.setup_inputs()))
    act = kernel(**inputs)
    err = np.abs(act - exp)
    print("absmax err:", err.max(), "rel:", err.max() / np.abs(exp).max())
